# revision 25
# baseline (speedup 1.0000x reference)
"""NT-Xent (SimCLR) contrastive loss on 8 Trainium2 NeuronCores.

Strategy (fully SPMD, no collectives, no DRAM staging):
  z = normalize(concat(emb_i, emb_j))  # [8192, 512]
  Each core c handles a 1024-row block of z. Inputs are pre-rotated on the
  host (np.roll by -c*1024 rows) and pre-cast to bf16 (halves input DMA);
  every core runs the identical program on rows 0..1023 of its own rotated
  copy: positive pair of rotated row i is rotated row (i + 4096) % 8192.

  Per core, per 1024-row octant:
    - DVE fused square+reduce (bf16 2x mode) -> sumsq
    - ACT rinv16 = exp(-0.5*ln(sumsq) + ln 16)  (scale 16 for fp8 range)
    - DVE+Pool scale rows to bf16 zbg = 16*z
    - PE identity-matmul transposes zbg chunks into a shared-PSUM tile
    - DVE copies PSUM -> SBUF bf16 (DoubleRow k-pair interleave layout)
    - SWDGE casting DMA converts bf16 -> fp8e4 into zT8[g] = [128, 2, 8192]
  Main loop: sim row-block via fp8 DoubleRow matmuls (2 insts per (m,
  n-tile), 256 contraction rows each) into [128, 4, 512] 4-bank PSUM tiles
  (tag shared with the transpose tiles, bufs=2); one ACT exp((2/256)*sim)
  per tile with free-dim accumulation -> row denominators.
  loss_row = ln(denom - exp(2*selfdot)) - 2*posdot; host gathers + means.
"""

import math

import ml_dtypes
import numpy as np

import concourse.bacc as bacc
import concourse.tile as tile
from concourse import mybir
from concourse.bass_utils import run_bass_kernel_spmd

N_CORES = 8
D = 512
ROWS = 8192
BLK = ROWS // N_CORES  # 1024
P = 128
GROUP = 8  # chunks per octant
NT = 512  # moving cols per matmul (one PSUM bank of fp32)
N_NT = ROWS // NT  # 16
NG = 4  # n-tiles fused per PSUM tile (4 banks)
KD = D // P  # 4 contraction k-tiles of 128
ZSCALE = 16.0  # fp8 operand scale; sim comes out scaled by ZSCALE^2
LN_ZSCALE = math.log(ZSCALE)
EXP_SCALE = 2.0 / (ZSCALE * ZSCALE)  # ACT scale turning psum into 2*sim

f32 = mybir.dt.float32
bf16 = mybir.dt.bfloat16
fp8 = mybir.dt.float8e4
i16 = mybir.dt.int16

_ACT_PATCHED = False


def _patch_act_tables():
    """Make Exp and Ln resolve only to natural_log_exp_and_others so the
    whole kernel uses a single activation-table set (one table load)."""
    global _ACT_PATCHED
    if _ACT_PATCHED:
        return
    import concourse.hw_specs as hw_specs

    Act = mybir.ActivationFunctionType
    orig = hw_specs.get_activation_tables("gen3")
    patched = {}
    for name, funcs in orig.items():
        fs = set(funcs)
        if name != "natural_log_exp_and_others":
            fs.discard(Act.Exp)
            fs.discard(Act.Ln)
        patched[name] = fs
    bacc.get_activation_tables = lambda arch: patched
    _ACT_PATCHED = True


def _build():
    _patch_act_tables()
    nc = bacc.Bacc("TRN2", target_bir_lowering=False)
    emb = nc.dram_tensor("emb", [ROWS, D], bf16, kind="ExternalInput")
    loss = nc.dram_tensor("loss", [P, GROUP], f32, kind="ExternalOutput")

    with tile.TileContext(nc) as tc:
        with (
            tc.tile_pool(name="persist", bufs=1) as persist,
            tc.tile_pool(name="loads", bufs=6) as loads,
            tc.tile_pool(name="zbgs", bufs=3) as zbgs,
            tc.tile_pool(name="ztgs", bufs=2) as ztgs,
            tc.tile_pool(name="scratch", bufs=3) as scratch,
            tc.tile_pool(name="small", bufs=2) as small,
            tc.tile_pool(name="psum", bufs=2, space="PSUM") as psum_pool,
        ):
            _body(nc, tc, persist, loads, zbgs, ztgs, scratch, small, psum_pool, emb, loss)

    nc.compile()
    return nc


def _body(nc, tc, persist, loads, zbgs, ztgs, scratch, small, psum_pool, emb, loss):
    Alu = mybir.AluOpType
    Act = mybir.ActivationFunctionType
    PM = mybir.MatmulPerfMode

    # persistent tensors
    # zT8[g][p, t, c] = fp8(16 * z[c, (2g+t)*128 + p]) -- DoubleRow operands
    zT8 = [
        persist.tile([P, 2, ROWS], fp8, tag=f"zT8_{g}", name=f"zT8_{g}")
        for g in range(2)
    ]
    acc = [persist.tile([P, 5], f32, tag=f"acc{m}", name=f"acc{m}") for m in range(GROUP)]
    posd = persist.tile([P, GROUP], f32, tag="posd")
    lnk = persist.tile([P, 1], f32, tag="lnk")  # bias const ln(ZSCALE)
    nc.gpsimd.memset(lnk, LN_ZSCALE)
    # identity for PE transposes: iota (f - p) == 0
    iom = persist.tile([P, P], i16, tag="iom")
    nc.gpsimd.iota(iom, pattern=[[1, P]], channel_multiplier=-1)
    ident = persist.tile([P, P], bf16, tag="ident")
    nc.vector.tensor_scalar(
        out=ident, in0=iom, scalar1=0, scalar2=None, op0=Alu.is_equal
    )
    zbg_keep = {}
    state = {}
    HC = GROUP // 2  # chunks per half-octant (4)

    def norm_half(h):
        """load + sumsq + rinv + row-scale for one half-octant (512 rows)."""
        oct_, hi = h // 2, h % 2
        et = loads.tile([P, HC, D], bf16, tag="et")
        r0 = h * HC * P
        src = emb[r0 : r0 + HC * P, :].rearrange("(c p) d -> p c d", p=P)
        nc.sync.dma_start(out=et, in_=src)
        if hi == 0:
            sq = small.tile([P, GROUP], f32, tag="sq", name=f"sq{oct_}")
            rinv = small.tile([P, GROUP], f32, tag="rinv", name=f"rinv{oct_}")
            state[("sq", oct_)] = sq
            state[("rinv", oct_)] = rinv
        else:
            sq = state[("sq", oct_)]
            rinv = state[("rinv", oct_)]
        for i in range(HC):
            tt = scratch.tile([P, D], bf16, tag="ttout")
            nc.vector.scalar_tensor_tensor(
                out=tt,
                in0=et[:, i, :],
                scalar=1.0,
                in1=et[:, i, :],
                op0=Alu.mult,
                op1=Alu.mult,
                accum_out=sq[:, hi * HC + i : hi * HC + i + 1],
            )
        # rinv16 = exp(-0.5*ln(sumsq) + ln 16); per-half for the fill-critical
        # first two octants, per-octant otherwise (fewer small ACT ops)
        sl = slice(hi * HC, (hi + 1) * HC)
        lnv = small.tile([P, HC], f32, tag="lnv")
        nc.scalar.activation(out=lnv, in_=sq[:, sl], func=Act.Ln)
        nc.scalar.activation(
            out=rinv[:, sl], in_=lnv, func=Act.Exp, scale=-0.5, bias=lnk[:, 0:1]
        )

        if hi == 0:
            if oct_ in (0, 4):
                zbg = persist.tile(
                    [P, GROUP, D], bf16, tag=f"zbg{oct_}", name=f"zbg{oct_}"
                )
                zbg_keep[oct_] = zbg
            else:
                zbg = zbgs.tile([P, GROUP, D], bf16, tag="zbg")
            state[oct_] = zbg
        zbg = state[oct_]
        # row-scaling on Pool (DVE is loaded with sumsq + psum copies)
        for i in range(HC):
            c = hi * HC + i
            nc.gpsimd.tensor_scalar_mul(
                out=zbg[:, c, :], in0=et[:, i, :], scalar1=rinv[:, c : c + 1]
            )

    def tr_half(h):
        """PE transposes + one DVE psum->sbuf copy for one half-octant."""
        oct_, hi = h // 2, h % 2
        pair = oct_ // 2
        zbg = state[oct_]
        tr = psum_pool.tile([P, HC, KD, P], bf16, tag="ps", bufs=2)
        for i in range(HC):
            for k in range(KD):
                nc.tensor.transpose(
                    tr[:, i, k, :], zbg[:, hi * HC + i, k * P : (k + 1) * P], ident
                )
        if ("ztp", pair) not in state:
            ztp = ztgs.tile([P, KD, 2 * GROUP, P], bf16, tag="ztp", name=f"ztp{pair}")
            state[("ztp", pair)] = ztp
        ztp = state[("ztp", pair)]
        q = (oct_ % 2) * GROUP + hi * HC
        # tr[p, i, k, c] -> ztp[p, k, q + i, c] in one permuted-AP copy
        nc.vector.tensor_copy(
            ztp[:, :, q : q + HC, :],
            tr.rearrange("p i k c -> p k i c"),
        )

    def cast_cols(pair, q0, q1):
        """SWDGE cast ztp columns [q0, q1) of a pair into zT8 (both g)."""
        ztp = state[("ztp", pair)]
        c0 = pair * 2 * BLK + q0 * P
        c1 = pair * 2 * BLK + q1 * P
        for g in range(2):
            nc.gpsimd.dma_start(
                out=zT8[g][:, :, c0:c1],
                in_=ztp[:, 2 * g : 2 * g + 2, q0:q1, :],
            )

    def selfposd():
        # positive-pair dots only: the self-similarity term subtracted from
        # the denominator is exp(2*||z||^2) = e^2 to ~1e-5 relative effect,
        # so a compile-time constant replaces the per-row self-dot.
        for m in range(GROUP):
            t2 = scratch.tile([P, D], bf16, tag="ttout")
            nc.vector.scalar_tensor_tensor(
                out=t2,
                in0=zbg_keep[0][:, m, :],
                scalar=1.0,
                in1=zbg_keep[4][:, m, :],
                op0=Alu.mult,
                op1=Alu.mult,
                accum_out=posd[:, m : m + 1],
            )

    def main_m(ng, nlist, m):
        """One PSUM tile: row-chunk m x n-tiles nlist, then ACT exp+accum."""
        w = len(nlist)
        pst = psum_pool.tile([P, w, NT], f32, tag="ps", bufs=2)
        for g in range(2):
            lhsT = zT8[g][:, :, m * P : (m + 1) * P]
            for li, n in enumerate(nlist):
                nc.tensor.matmul(
                    pst[:, li, :],
                    lhsT,
                    zT8[g][:, :, n * NT : (n + 1) * NT],
                    start=(g == 0),
                    stop=(g == 1),
                    perf_mode=PM.DoubleRow,
                )
        ex = scratch.tile([P, w, NT], bf16, tag="exout")
        nc.scalar.activation(
            out=ex,
            in_=pst,
            func=Act.Exp,
            scale=EXP_SCALE,
            accum_out=acc[m][:, ng : ng + 1],
        )

    def finale():
        dsum = persist.tile([P, GROUP], f32, tag="dsum")
        for m in range(GROUP):
            nc.vector.reduce_sum(
                out=dsum[:, m : m + 1], in_=acc[m], axis=mybir.AxisListType.X
            )
        dx = small.tile([P, GROUP], f32, tag="dx")
        nc.vector.tensor_scalar_add(out=dx, in0=dsum, scalar1=-math.exp(2.0))
        ld = small.tile([P, GROUP], f32, tag="ld")
        nc.scalar.activation(out=ld, in_=dx, func=Act.Ln)
        lossv = small.tile([P, GROUP], f32, tag="lossv")
        nc.vector.scalar_tensor_tensor(
            out=lossv,
            in0=posd,
            scalar=-EXP_SCALE,
            in1=ld,
            op0=Alu.mult,
            op1=Alu.add,
        )
        nc.sync.dma_start(out=loss[:, :], in_=lossv)

    # Explicit emission schedule. Engine program order == emission order, so
    # normalize / transpose / cast work for octant pair p+1 is threaded
    # between the matmul+exp tiles of group p; PSUM 'ps' slots rotate across
    # both transpose tiles and matmul tiles.
    GROUPS = [[0, 1], [2, 3], [4, 5, 6, 7], [8, 9, 10, 11], [12, 13, 14, 15]]
    # fill: octants 0 and 1 at half-octant granularity, then group 0/1
    norm_half(0)
    norm_half(1)
    tr_half(0)
    tr_half(1)
    cast_cols(0, 0, GROUP)  # oct0
    norm_half(2)
    norm_half(3)
    norm_half(4)
    norm_half(5)
    # G0 (n-tiles 0-1, needs oct0); thread oct1 + pair-1 transposes in
    for m in range(GROUP):
        main_m(0, GROUPS[0], m)
        if m == 0:
            tr_half(2)
        if m == 1:
            tr_half(3)
        if m == 2:
            cast_cols(0, GROUP, 2 * GROUP)  # oct1
        if m == 3:
            tr_half(4)
        if m == 4:
            norm_half(6)
            tr_half(5)
        if m == 5:
            norm_half(7)
    # G1 (n-tiles 2-3, needs oct1)
    for m in range(GROUP):
        main_m(1, GROUPS[1], m)
        if m == 0:
            tr_half(6)
        if m == 1:
            cast_cols(1, 0, GROUP)  # oct2
        if m == 2:
            tr_half(7)
        if m == 3:
            cast_cols(1, GROUP, 2 * GROUP)  # oct3
        if m == 4:
            norm_half(8)
        if m == 6:
            norm_half(9)
    # G2 (n-tiles 4-7, needs octants 2-3); prepare pair 2 (octants 4-5)
    for m in range(GROUP):
        main_m(2, GROUPS[2], m)
        if m == 0:
            tr_half(8)
        if m == 1:
            norm_half(10)
        if m == 2:
            tr_half(9)
        if m == 3:
            norm_half(11)
        if m == 4:
            tr_half(10)
        if m == 5:
            tr_half(11)
            selfposd()
        if m == 6:
            cast_cols(2, 0, 2 * GROUP)  # octants 4-5
            norm_half(12)
        if m == 7:
            norm_half(13)
    # G3 (n-tiles 8-11, needs octants 4-5); prepare pair 3 (octants 6-7)
    for m in range(GROUP):
        main_m(3, GROUPS[3], m)
        if m == 0:
            tr_half(12)
        if m == 1:
            norm_half(14)
        if m == 2:
            tr_half(13)
        if m == 3:
            norm_half(15)
        if m == 4:
            tr_half(14)
        if m == 5:
            tr_half(15)
        if m == 6:
            cast_cols(3, 0, 2 * GROUP)  # octants 6-7
    # G4 (n-tiles 12-15, needs octants 6-7)
    for m in range(GROUP):
        main_m(4, GROUPS[4], m)
    finale()


_NC_CACHE = []


def _get_nc():
    if not _NC_CACHE:
        _NC_CACHE.append(_build())
    return _NC_CACHE[0]


def make_in_maps(emb_i: np.ndarray, emb_j: np.ndarray):
    emb_all = np.concatenate(
        [np.asarray(emb_i, np.float32), np.asarray(emb_j, np.float32)], axis=0
    ).astype(ml_dtypes.bfloat16)
    return [
        {"emb": np.ascontiguousarray(np.roll(emb_all, -c * BLK, axis=0))}
        for c in range(N_CORES)
    ]


def assemble(results) -> np.ndarray:
    rows = []
    for c in range(N_CORES):
        out = results[c]["loss"]  # [128, 8]; out[p, m] = loss of block row m*128+p
        rows.append(out.T.reshape(-1))
    all_rows = np.concatenate(rows)  # original row order
    return np.float32(all_rows.astype(np.float64).mean())


def kernel(emb_i: np.ndarray, emb_j: np.ndarray) -> np.ndarray:
    nc = _get_nc()
    res = run_bass_kernel_spmd(
        nc, make_in_maps(emb_i, emb_j), core_ids=list(range(N_CORES))
    )
    return assemble(res.results)


if __name__ == "__main__":
    rng = np.random.default_rng(0)
    ei = rng.standard_normal((4096, D)).astype(np.float32)
    ej = rng.standard_normal((4096, D)).astype(np.float32)
    print(kernel(ei, ej))
